# revision 10
# baseline (speedup 1.0000x reference)
"""Trainium2 Bass kernel for a 2-layer LSTM decoder VAE head.

Strategy: 8-way tensor parallelism over the hidden dim (H=1024 -> 128 rows per
core); all state kept transposed ([feature, batch]) so no transposes are ever
needed; the output MLP is replicated on every core (cheaper than an AllReduce
of its tiny result).  Per step each core exchanges its h1/h2 chunks with the
other cores via AllGather.

The batch B=256 is split into two independent 128-wide chains whose step loops
are interleaved: while one chain waits for its AllGather to land, the PE runs
the other chain's matmuls.  This keeps the tensor engine busy (HAM stays at
full clock) and the 64KB/rank payload uses the fast one-hop Mesh collective.

Self-contained: only needs numpy + the concourse (Bass/Tile) runtime that is
preinstalled on the machine.
"""

import os
import numpy as np

B, SEQ, H, COORD = 256, 200, 1024, 8
LATS = (32, 64, 128)
TOT = sum(LATS)  # 224
N_CORES = 8
HC = H // N_CORES  # 128 rows per core
KT = H // 128      # 8 K tiles
NB = B // 2        # batch per chain

_CACHE = {}


def _mmdt():
    return os.environ.get("BASS_KERNEL_MMDT", "bf16")


def _build(seq, mmdt):
    import concourse.bass as bass
    import concourse.tile as tile
    from concourse import bacc, mybir

    f32 = mybir.dt.float32
    DT = {"fp32": mybir.dt.float32, "bf16": mybir.dt.bfloat16,
          "fp32r": mybir.dt.float32r}[mmdt]
    AF = mybir.ActivationFunctionType

    nc = bacc.Bacc("TRN2", target_bir_lowering=False, debug=False,
                   num_devices=N_CORES)

    def din(name, shape, dt=None):
        return nc.dram_tensor(name, list(shape), dt or f32,
                              kind="ExternalInput")

    whh0 = din("whh0", (128, 4, KT, 128), DT)
    wih1 = din("wih1", (128, 4, KT, 128), DT)
    whh1 = din("whh1", (128, 4, KT, 128), DT)
    wo1 = din("wo1", (128, 4, KT, 128), DT)
    wih0 = din("wih0", (8, 4, 128), DT)
    wo2 = din("wo2", (128, 4, 8), DT)
    wproj = din("wproj", (128, 9, 2, 128))
    zt = din("zt", (128, 2, B))
    bg0 = din("bg0", (128, 4))
    bg1 = din("bg1", (128, 4))
    bo1 = din("bo1", (128, 4))
    bo2 = din("bo2", (8, 1))
    bproj = din("bproj", (128, 9))

    OUT = nc.dram_tensor("out", [seq, 8, B], f32, kind="ExternalOutput")

    def persist(name, shape, dtype=f32):
        return nc.alloc_sbuf_tensor(name, list(shape), dtype).ap()

    whh0_sb = persist("whh0_sb", [128, 4, KT, 128], DT)
    wih1_sb = persist("wih1_sb", [128, 4, KT, 128], DT)
    whh1_sb = persist("whh1_sb", [128, 4, KT, 128], DT)
    wo1_sb = persist("wo1_sb", [128, 4, KT, 128], DT)
    wih0_sb = persist("wih0_sb", [8, 4, 128], DT)
    wo2_sb = persist("wo2_sb", [128, 4, 8], DT)
    wproj_sb = persist("wproj_sb", [128, 9, 2, 128])
    zt_sb = persist("zt_sb", [128, 2, B])
    bg0_sb = persist("bg0_sb", [128, 4])
    bg1_sb = persist("bg1_sb", [128, 4])
    bo1_sb = persist("bo1_sb", [128, 4])
    bo2_sb = persist("bo2_sb", [8, 1])
    bproj_sb = persist("bproj_sb", [128, 9])

    class Chain:
        def __init__(self, name, b0):
            self.name = name
            self.b0 = b0                      # batch offset into OUT
            self.h1T = persist(f"h1T_{name}", [128, KT, NB], DT)
            self.h2T = persist(f"h2T_{name}", [128, KT, NB], DT)
            self.c1 = persist(f"c1_{name}", [128, NB])
            self.c2 = persist(f"c2_{name}", [128, NB])
            self.xT = persist(f"xT_{name}", [8, NB], DT)
            self.xTf = persist(f"xTf_{name}", [8, NB])
            self.g0 = None                    # open gates0 psum tiles

    with tile.TileContext(nc) as tc:
        A = Chain("a", 0)
        Bc = Chain("b", NB)
        chains = (A, Bc)

        for dst, src in (
            (whh0_sb, whh0), (wih1_sb, wih1), (whh1_sb, whh1), (wo1_sb, wo1),
            (wih0_sb, wih0), (wo2_sb, wo2), (wproj_sb, wproj), (zt_sb, zt),
            (bg0_sb, bg0), (bg1_sb, bg1), (bo1_sb, bo1), (bo2_sb, bo2),
            (bproj_sb, bproj),
        ):
            nc.sync.dma_start(dst[:], src.ap())

        with (
            tc.tile_pool(name="psg", bufs=6, space="PSUM") as psg,
            tc.tile_pool(name="psm", bufs=2, space="PSUM") as psm,
            tc.tile_pool(name="nl", bufs=3) as nl,
            tc.tile_pool(name="dram", bufs=2, space="DRAM") as dram,
        ):
            # ---- init: h0 full + own c0 chunk via W_proj (both chains) ----
            for m in range(9):
                ps = psg.tile([128, B], f32, name="ps_init", tag="g")
                nc.tensor.matmul(ps[:], wproj_sb[:, m, 0, :], zt_sb[:, 0, :],
                                 start=True, stop=False)
                nc.tensor.matmul(ps[:], wproj_sb[:, m, 1, :], zt_sb[:, 1, :],
                                 start=False, stop=True)
                for ch in chains:
                    sl = ps[:, ch.b0:ch.b0 + NB]
                    if m < 8:
                        nc.scalar.activation(ch.h1T[:, m, :], sl, AF.Identity,
                                             bias=bproj_sb[:, m:m + 1])
                        nc.vector.tensor_copy(ch.h2T[:, m, :], ch.h1T[:, m, :])
                    else:
                        nc.scalar.activation(ch.c1[:, :], sl, AF.Identity,
                                             bias=bproj_sb[:, m:m + 1])
                        nc.vector.tensor_copy(ch.c2[:, :], ch.c1[:, :])

            # ---- gates0(0): Whh0 @ h0 (x(-1) = 0, no Wih0 term) ----
            for ch in chains:
                ch.g0 = []
                for g in range(4):
                    ps = psg.tile([128, NB], f32, name="ps_g0", tag="g")
                    for k in range(KT):
                        nc.tensor.matmul(ps[:], whh0_sb[:, g, k, :],
                                         ch.h1T[:, k, :],
                                         start=(k == 0), stop=(k == KT - 1))
                    ch.g0.append(ps)

            ND_WARM = int(os.environ.get("BASS_KERNEL_NDWARM", "16"))

            def warm_pe(nwarm):
                # dummy matmuls on resident weights: keeps the PE HAM
                # activity monitor busy across an AllGather stall so the
                # clock stays at 2.4GHz; results are never read.
                if nwarm <= 0:
                    return
                dps = psm.tile([128, NB], f32, name="ps_warm", tag="m")
                for i in range(nwarm):
                    nc.tensor.matmul(dps[:], whh1_sb[:, 0, i % KT, :],
                                     whh1_sb[:, 1, i % KT, :],
                                     start=True, stop=True)

            def lstm_nonlin(ch, gps, c_sb, bias_sb, lay):
                sfx = f"{ch.name}{lay}"
                sig_i = nl.tile([128, NB], f32, name="sig_i", tag=f"si_{sfx}")
                sig_f = nl.tile([128, NB], f32, name="sig_f", tag=f"sf_{sfx}")
                tan_g = nl.tile([128, NB], f32, name="tan_g", tag=f"tg_{sfx}")
                sig_o = nl.tile([128, NB], f32, name="sig_o", tag=f"so_{sfx}")
                nc.scalar.activation(sig_i[:], gps[0][:], AF.Sigmoid,
                                     bias=bias_sb[:, 0:1])
                nc.scalar.activation(sig_f[:], gps[1][:], AF.Sigmoid,
                                     bias=bias_sb[:, 1:2])
                nc.scalar.activation(tan_g[:], gps[2][:], AF.Tanh,
                                     bias=bias_sb[:, 2:3])
                nc.scalar.activation(sig_o[:], gps[3][:], AF.Sigmoid,
                                     bias=bias_sb[:, 3:4])
                t_fc = nl.tile([128, NB], f32, name="t_fc", tag=f"fc_{sfx}")
                t_ig = nl.tile([128, NB], f32, name="t_ig", tag=f"ig_{sfx}")
                nc.vector.tensor_mul(t_fc[:], sig_f[:], c_sb[:, :])
                nc.vector.tensor_mul(t_ig[:], sig_i[:], tan_g[:])
                nc.vector.tensor_add(c_sb[:, :], t_fc[:], t_ig[:])
                tan_c = nl.tile([128, NB], f32, name="tan_c", tag=f"tc_{sfx}")
                nc.scalar.activation(tan_c[:], c_sb[:, :], AF.Tanh)
                hch = nl.tile([128, NB], DT, name="hch", tag=f"h_{sfx}")
                nc.vector.tensor_mul(hch[:], sig_o[:], tan_c[:])
                return hch

            def gather(ch, hch, dest, lay):
                sfx = f"{ch.name}{lay}"
                inb = dram.tile([128, NB], DT, name="agin", tag=f"agi_{sfx}")
                outb = dram.tile([128 * N_CORES, NB], DT, name="agout",
                                 tag=f"ago_{sfx}", addr_space="Shared")
                nc.sync.dma_start(inb[:], hch[:])
                nc.gpsimd.collective_compute(
                    "AllGather", mybir.AluOpType.bypass,
                    replica_groups=[list(range(N_CORES))],
                    ins=[inb.opt()], outs=[outb.opt()],
                )
                for eng, k0, nk in ((nc.sync, 0, 1), (nc.scalar, 1, 1),
                                    (nc.sync, 2, 3), (nc.scalar, 5, 3)):
                    eng.dma_start(
                        dest[:, k0:k0 + nk, :],
                        outb[k0 * 128:(k0 + nk) * 128, :].rearrange(
                            "(k p) n -> p k n", p=128))

            # per-chain emission pieces -----------------------------------
            def emit_front(ch, t):
                """Whh1 (ready work), then Wih1 (stalls on AG1), then the
                layer-1 nonlinearity and the h2 AllGather launch."""
                g1 = []
                for g in range(4):
                    ps = psg.tile([128, NB], f32, name="ps_g1", tag="g")
                    for k in range(KT):
                        nc.tensor.matmul(ps[:], whh1_sb[:, g, k, :],
                                         ch.h2T[:, k, :],
                                         start=(k == 0), stop=False)
                    g1.append(ps)
                warm_pe(ND_WARM)
                for k in range(KT):
                    for g in range(4):
                        nc.tensor.matmul(g1[g][:], wih1_sb[:, g, k, :],
                                         ch.h1T[:, k, :],
                                         start=False, stop=(k == KT - 1))
                h2ch = lstm_nonlin(ch, g1, ch.c2, bg1_sb, 1)
                gather(ch, h2ch, ch.h2T, 1)

            def emit_back(ch, t):
                """Whh0(t+1) (ready after AG1), MLP(t) (stalls on AG2),
                Wih0(t+1), then the layer-0 nonlinearity for t+1 and the h1
                AllGather launch."""
                last = t == seq - 1
                if not last:
                    g0n = [psg.tile([128, NB], f32, name=f"ps_g0{g}", tag="g")
                           for g in range(4)]
                    for k in range(KT):
                        for g in range(4):
                            nc.tensor.matmul(g0n[g][:], whh0_sb[:, g, k, :],
                                             ch.h1T[:, k, :],
                                             start=(k == 0), stop=False)
                warm_pe(ND_WARM)
                relu = nl.tile([128, 4, NB], DT, name="relu",
                               tag=f"relu_{ch.name}")
                for m in range(4):
                    ps = psm.tile([128, NB], f32, name="ps_mlp", tag="m")
                    for k in range(KT):
                        nc.tensor.matmul(ps[:], wo1_sb[:, m, k, :],
                                         ch.h2T[:, k, :],
                                         start=(k == 0), stop=(k == KT - 1))
                    nc.scalar.activation(relu[:, m, :], ps[:], AF.Relu,
                                         bias=bo1_sb[:, m:m + 1])
                ps_x = psm.tile([8, NB], f32, name="ps_x", tag="m")
                for k in range(4):
                    nc.tensor.matmul(ps_x[:], wo2_sb[:, k, :], relu[:, k, :],
                                     start=(k == 0), stop=(k == 3))
                nc.scalar.activation(ch.xTf[:, :], ps_x[:], AF.Identity,
                                     bias=bo2_sb[:, 0:1])
                nc.scalar.activation(ch.xT[:, :], ps_x[:], AF.Identity,
                                     bias=bo2_sb[:, 0:1])
                nc.sync.dma_start(OUT.ap()[t][:, ch.b0:ch.b0 + NB],
                                  ch.xTf[:, :])
                if not last:
                    for g in range(4):
                        nc.tensor.matmul(g0n[g][:], wih0_sb[:, g, :],
                                         ch.xT[:, :], start=False, stop=True)
                    h1ch = lstm_nonlin(ch, g0n, ch.c1, bg0_sb, 0)
                    gather(ch, h1ch, ch.h1T, 0)

            # kick off step 0's layer-0 nonlinearity + h1 gathers
            for ch in chains:
                h1ch = lstm_nonlin(ch, ch.g0, ch.c1, bg0_sb, 0)
                gather(ch, h1ch, ch.h1T, 0)

            for t in range(seq):
                for ch in chains:
                    emit_front(ch, t)
                for ch in chains:
                    emit_back(ch, t)

    nc.compile()
    return nc


def _lhsT_tiles(W, rows, K):
    """W[rows] viewed as lhsT tiles: [128, MT, KTl, 128] with
    out[ki, mt, kt, mi] = W[rows[mt*128+mi], kt*128+ki]."""
    R = len(rows)
    MT = R // 128
    KTl = K // 128
    t = W[rows].reshape(MT, 128, KTl, 128)          # [mt, mi, kt, ki]
    return np.ascontiguousarray(t.transpose(3, 0, 2, 1)).astype(np.float32)


def _prep_inputs(inputs):
    import ml_dtypes
    np_dt = {"fp32": np.float32, "bf16": ml_dtypes.bfloat16,
             "fp32r": np.float32}[_mmdt()]
    f = lambda k: np.asarray(inputs[k], np.float32)
    W_proj, b_proj = f("W_proj"), f("b_proj")
    W_ih0, W_hh0 = f("W_ih0"), f("W_hh0")
    b_ih0, b_hh0 = f("b_ih0"), f("b_hh0")
    W_ih1, W_hh1 = f("W_ih1"), f("W_hh1")
    b_ih1, b_hh1 = f("b_ih1"), f("b_hh1")
    W_o1, b_o1 = f("W_o1"), f("b_o1")
    W_o2, b_o2 = f("W_o2"), f("b_o2")
    z = np.concatenate([f("z_primitive"), f("z_skill"), f("z_style")], axis=1)

    wo1 = _lhsT_tiles(W_o1, np.arange(512), H)
    wo2 = np.ascontiguousarray(
        W_o2.T.reshape(4, 128, 8).transpose(1, 0, 2)).astype(np.float32)
    bo1 = np.ascontiguousarray(b_o1.reshape(4, 128).T).astype(np.float32)
    bo2 = b_o2.reshape(8, 1).astype(np.float32)
    ztp = np.zeros((256, B), np.float32)
    ztp[:TOT] = z.T
    zt = np.ascontiguousarray(
        ztp.reshape(2, 128, B).transpose(1, 0, 2))
    Wp = np.zeros((2 * H, 256), np.float32)
    Wp[:, :TOT] = W_proj
    bias_g0 = b_ih0 + b_hh0
    bias_g1 = b_ih1 + b_hh1

    in_maps = []
    for c in range(N_CORES):
        rows_g = np.concatenate(
            [g * H + c * HC + np.arange(HC) for g in range(4)])
        rows_p = np.concatenate([np.arange(H), H + c * HC + np.arange(HC)])
        wih0 = np.ascontiguousarray(
            W_ih0[rows_g].reshape(4, 128, 8).transpose(2, 0, 1)).astype(
                np.float32)
        in_maps.append({
            "whh0": _lhsT_tiles(W_hh0, rows_g, H).astype(np_dt),
            "wih1": _lhsT_tiles(W_ih1, rows_g, H).astype(np_dt),
            "whh1": _lhsT_tiles(W_hh1, rows_g, H).astype(np_dt),
            "wo1": wo1.astype(np_dt),
            "wih0": wih0.astype(np_dt),
            "wo2": wo2.astype(np_dt),
            "wproj": _lhsT_tiles(Wp, rows_p, 256),
            "zt": zt,
            "bg0": np.ascontiguousarray(
                bias_g0[rows_g].reshape(4, 128).T).astype(np.float32),
            "bg1": np.ascontiguousarray(
                bias_g1[rows_g].reshape(4, 128).T).astype(np.float32),
            "bo1": bo1,
            "bo2": bo2,
            "bproj": np.ascontiguousarray(
                b_proj[rows_p].reshape(9, 128).T).astype(np.float32),
        })
    return in_maps


def kernel(**inputs):
    from concourse.bass_utils import run_bass_kernel_spmd

    seq = int(os.environ.get("BASS_KERNEL_SEQ", SEQ))
    key = (seq, _mmdt())
    if key not in _CACHE:
        _CACHE[key] = _build(seq, _mmdt())
    nc = _CACHE[key]
    in_maps = _prep_inputs(inputs)

    trace = os.environ.get("BASS_KERNEL_TRACE", "") == "1"
    kwargs = {}
    if trace:
        kwargs["trace"] = True
        kwargs["tmpdir"] = os.environ.get("BASS_KERNEL_TRACE_DIR") or None
    res = run_bass_kernel_spmd(nc, in_maps, core_ids=list(range(N_CORES)),
                               **kwargs)
    if trace:
        kernel.last_exec_time_ns = res.exec_time_ns
    out = res.results[0]["out"]          # [seq, 8, B]
    return np.ascontiguousarray(out.transpose(2, 0, 1)).astype(np.float32)


kernel.last_exec_time_ns = None



# revision 15
# speedup vs baseline: 1.1077x; 1.1077x over previous
"""Trainium2 Bass kernel for a 2-layer LSTM decoder VAE head.

Strategy: 8-way tensor parallelism over the hidden dim (H=1024 -> 128 rows per
core); all state kept transposed ([feature, batch]) so no transposes are ever
needed; the output MLP is replicated on every core (cheaper than an AllReduce
of its tiny result).  Per step each core exchanges its h1/h2 chunks with the
other cores via AllGather.

The batch B=256 is split into two independent 128-wide chains whose step loops
are interleaved: while one chain waits for its AllGather to land, the PE runs
the other chain's matmuls.  This keeps the tensor engine busy (HAM stays at
full clock) and the 64KB/rank payload uses the fast one-hop Mesh collective.

Self-contained: only needs numpy + the concourse (Bass/Tile) runtime that is
preinstalled on the machine.
"""

import os
import numpy as np

B, SEQ, H, COORD = 256, 200, 1024, 8
LATS = (32, 64, 128)
TOT = sum(LATS)  # 224
N_CORES = 8
HC = H // N_CORES  # 128 rows per core
KT = H // 128      # 8 K tiles
NB = B // 2        # batch per chain

_CACHE = {}


def _mmdt():
    return os.environ.get("BASS_KERNEL_MMDT", "bf16")


def _build(seq, mmdt):
    import concourse.bass as bass
    import concourse.tile as tile
    from concourse import bacc, mybir

    f32 = mybir.dt.float32
    DT = {"fp32": mybir.dt.float32, "bf16": mybir.dt.bfloat16,
          "fp32r": mybir.dt.float32r}[mmdt]
    AF = mybir.ActivationFunctionType

    nc = bacc.Bacc("TRN2", target_bir_lowering=False, debug=False,
                   num_devices=N_CORES)

    def din(name, shape, dt=None):
        return nc.dram_tensor(name, list(shape), dt or f32,
                              kind="ExternalInput")

    whh0 = din("whh0", (128, 4, KT, 128), DT)
    wih1 = din("wih1", (128, 4, KT, 128), DT)
    whh1 = din("whh1", (128, 4, KT, 128), DT)
    wo1 = din("wo1", (128, 4, KT, 128), DT)
    wih0 = din("wih0", (8, 4, 128), DT)
    wo2 = din("wo2", (128, 4, 8), DT)
    wproj = din("wproj", (128, 9, 2, 128))
    zt = din("zt", (128, 2, B))
    bg0 = din("bg0", (128, 4))
    bg1 = din("bg1", (128, 4))
    bo1 = din("bo1", (128, 4))
    bo2 = din("bo2", (8, 1))
    bproj = din("bproj", (128, 9))

    OUT = nc.dram_tensor("out", [seq, 8, B], f32, kind="ExternalOutput")

    def persist(name, shape, dtype=f32):
        return nc.alloc_sbuf_tensor(name, list(shape), dtype).ap()

    whh0_sb = persist("whh0_sb", [128, 4, KT, 128], DT)
    wih1_sb = persist("wih1_sb", [128, 4, KT, 128], DT)
    whh1_sb = persist("whh1_sb", [128, 4, KT, 128], DT)
    wo1_sb = persist("wo1_sb", [128, 4, KT, 128], DT)
    wih0_sb = persist("wih0_sb", [8, 4, 128], DT)
    wo2_sb = persist("wo2_sb", [128, 4, 8], DT)
    wproj_sb = persist("wproj_sb", [128, 9, 2, 128])
    zt_sb = persist("zt_sb", [128, 2, B])
    bg0_sb = persist("bg0_sb", [128, 4])
    bg1_sb = persist("bg1_sb", [128, 4])
    bo1_sb = persist("bo1_sb", [128, 4])
    bo2_sb = persist("bo2_sb", [8, 1])
    bproj_sb = persist("bproj_sb", [128, 9])

    class Chain:
        def __init__(self, name, b0):
            self.name = name
            self.b0 = b0                      # batch offset into OUT
            self.h1T = persist(f"h1T_{name}", [128, KT, NB], DT)
            self.h2T = persist(f"h2T_{name}", [128, KT, NB], DT)
            self.c1 = persist(f"c1_{name}", [128, NB])
            self.c2 = persist(f"c2_{name}", [128, NB])
            self.xT = persist(f"xT_{name}", [8, NB], DT)
            self.xTf = persist(f"xTf_{name}", [8, NB])
            self.g0 = None                    # open gates0 psum tiles

    with tile.TileContext(nc) as tc:
        A = Chain("a", 0)
        Bc = Chain("b", NB)
        chains = (A, Bc)

        for dst, src in (
            (whh0_sb, whh0), (wih1_sb, wih1), (whh1_sb, whh1), (wo1_sb, wo1),
            (wih0_sb, wih0), (wo2_sb, wo2), (wproj_sb, wproj), (zt_sb, zt),
            (bg0_sb, bg0), (bg1_sb, bg1), (bo1_sb, bo1), (bo2_sb, bo2),
            (bproj_sb, bproj),
        ):
            nc.sync.dma_start(dst[:], src.ap())

        with (
            tc.tile_pool(name="psg", bufs=6, space="PSUM") as psg,
            tc.tile_pool(name="psm", bufs=2, space="PSUM") as psm,
            tc.tile_pool(name="nl", bufs=3) as nl,
            tc.tile_pool(name="dram", bufs=2, space="DRAM") as dram,
        ):
            # ---- init: h0 full + own c0 chunk via W_proj (both chains) ----
            for m in range(9):
                ps = psg.tile([128, B], f32, name="ps_init", tag="g")
                nc.tensor.matmul(ps[:], wproj_sb[:, m, 0, :], zt_sb[:, 0, :],
                                 start=True, stop=False)
                nc.tensor.matmul(ps[:], wproj_sb[:, m, 1, :], zt_sb[:, 1, :],
                                 start=False, stop=True)
                for ch in chains:
                    sl = ps[:, ch.b0:ch.b0 + NB]
                    if m < 8:
                        nc.scalar.activation(ch.h1T[:, m, :], sl, AF.Identity,
                                             bias=bproj_sb[:, m:m + 1])
                        nc.vector.tensor_copy(ch.h2T[:, m, :], ch.h1T[:, m, :])
                    else:
                        nc.scalar.activation(ch.c1[:, :], sl, AF.Identity,
                                             bias=bproj_sb[:, m:m + 1])
                        nc.vector.tensor_copy(ch.c2[:, :], ch.c1[:, :])

            # ---- gates0(0): Whh0 @ h0 (x(-1) = 0, no Wih0 term) ----
            for ch in chains:
                ch.g0 = [None] * 4
                for g in (2, 0, 1, 3):
                    ps = psg.tile([128, NB], f32, name="ps_g0", tag="g")
                    for k in range(KT):
                        nc.tensor.matmul(ps[:], whh0_sb[:, g, k, :],
                                         ch.h1T[:, k, :],
                                         start=(k == 0), stop=(k == KT - 1))
                    ch.g0[g] = ps

            ND_WARM = int(os.environ.get("BASS_KERNEL_NDWARM", "16"))

            def warm_pe(nwarm):
                # dummy matmuls on resident weights: keeps the PE HAM
                # activity monitor busy across an AllGather stall so the
                # clock stays at 2.4GHz; results are never read.
                if nwarm <= 0:
                    return
                dps = psm.tile([128, NB], f32, name="ps_warm", tag="m")
                for i in range(nwarm):
                    nc.tensor.matmul(dps[:], whh1_sb[:, 0, i % KT, :],
                                     whh1_sb[:, 1, i % KT, :],
                                     start=True, stop=True)

            def lstm_nonlin(ch, gps, c_sb, bias_sb, lay):
                """Feeding matmuls are emitted gate-major in (g,i,f,o)
                order, so the g/i/f chain and the c update complete under
                the PE's trailing o-matmuls; the tail after the last
                o-matmul is ~sig_o + h-mul."""
                sfx = f"{ch.name}{lay}"
                sig_i = nl.tile([128, NB], f32, name="sig_i", tag=f"si_{sfx}")
                sig_f = nl.tile([128, NB], f32, name="sig_f", tag=f"sf_{sfx}")
                tan_g = nl.tile([128, NB], f32, name="tan_g", tag=f"tg_{sfx}")
                sig_o = nl.tile([128, NB], f32, name="sig_o", tag=f"so_{sfx}")
                nc.scalar.activation(tan_g[:], gps[2][:], AF.Tanh,
                                     bias=bias_sb[:, 2:3])
                nc.scalar.activation(sig_i[:], gps[0][:], AF.Sigmoid,
                                     bias=bias_sb[:, 0:1])
                nc.scalar.activation(sig_f[:], gps[1][:], AF.Sigmoid,
                                     bias=bias_sb[:, 1:2])
                t_fc = nl.tile([128, NB], f32, name="t_fc", tag=f"fc_{sfx}")
                t_ig = nl.tile([128, NB], f32, name="t_ig", tag=f"ig_{sfx}")
                nc.vector.tensor_mul(t_ig[:], sig_i[:], tan_g[:])
                nc.vector.tensor_mul(t_fc[:], sig_f[:], c_sb[:, :])
                nc.vector.tensor_add(c_sb[:, :], t_fc[:], t_ig[:])
                tan_c = nl.tile([128, NB], f32, name="tan_c", tag=f"tc_{sfx}")
                nc.scalar.activation(tan_c[:], c_sb[:, :], AF.Tanh)
                nc.scalar.activation(sig_o[:], gps[3][:], AF.Sigmoid,
                                     bias=bias_sb[:, 3:4])
                hch = nl.tile([128, NB], DT, name="hch", tag=f"h_{sfx}")
                nc.vector.tensor_mul(hch[:], sig_o[:], tan_c[:])
                return hch

            def gather(ch, hch, dest, lay):
                sfx = f"{ch.name}{lay}"
                inb = dram.tile([128, NB], DT, name="agin", tag=f"agi_{sfx}")
                outb = dram.tile([128 * N_CORES, NB], DT, name="agout",
                                 tag=f"ago_{sfx}", addr_space="Shared")
                nc.sync.dma_start(inb[:], hch[:])
                nc.gpsimd.collective_compute(
                    "AllGather", mybir.AluOpType.bypass,
                    replica_groups=[list(range(N_CORES))],
                    ins=[inb.opt()], outs=[outb.opt()],
                )
                for eng, k0, nk in ((nc.sync, 0, 1), (nc.scalar, 1, 1),
                                    (nc.sync, 2, 3), (nc.scalar, 5, 3)):
                    eng.dma_start(
                        dest[:, k0:k0 + nk, :],
                        outb[k0 * 128:(k0 + nk) * 128, :].rearrange(
                            "(k p) n -> p k n", p=128))

            # per-chain emission pieces -----------------------------------
            def emit_front(ch, t):
                """Whh1 (ready work), then Wih1 (stalls on AG1), then the
                layer-1 nonlinearity and the h2 AllGather launch."""
                g1 = [None] * 4
                for g in (2, 0, 1, 3):
                    ps = psg.tile([128, NB], f32, name="ps_g1", tag="g")
                    for k in range(KT):
                        nc.tensor.matmul(ps[:], whh1_sb[:, g, k, :],
                                         ch.h2T[:, k, :],
                                         start=(k == 0), stop=False)
                    g1[g] = ps
                warm_pe(ND_WARM)
                for g in (2, 0, 1, 3):
                    for k in range(KT):
                        nc.tensor.matmul(g1[g][:], wih1_sb[:, g, k, :],
                                         ch.h1T[:, k, :],
                                         start=False, stop=(k == KT - 1))
                h2ch = lstm_nonlin(ch, g1, ch.c2, bg1_sb, 1)
                gather(ch, h2ch, ch.h2T, 1)

            def emit_back(ch, t):
                """Whh0(t+1) (ready after AG1), MLP(t) (stalls on AG2),
                Wih0(t+1), then the layer-0 nonlinearity for t+1 and the h1
                AllGather launch."""
                last = t == seq - 1
                if not last:
                    g0n = [psg.tile([128, NB], f32, name=f"ps_g0{g}", tag="g")
                           for g in range(4)]
                    for g in (2, 0, 1, 3):
                        for k in range(KT):
                            nc.tensor.matmul(g0n[g][:], whh0_sb[:, g, k, :],
                                             ch.h1T[:, k, :],
                                             start=(k == 0), stop=False)
                warm_pe(ND_WARM)
                relu = nl.tile([128, 4, NB], DT, name="relu",
                               tag=f"relu_{ch.name}")
                for m in range(4):
                    ps = psm.tile([128, NB], f32, name="ps_mlp", tag="m")
                    for k in range(KT):
                        nc.tensor.matmul(ps[:], wo1_sb[:, m, k, :],
                                         ch.h2T[:, k, :],
                                         start=(k == 0), stop=(k == KT - 1))
                    nc.scalar.activation(relu[:, m, :], ps[:], AF.Relu,
                                         bias=bo1_sb[:, m:m + 1])
                ps_x = psm.tile([8, NB], f32, name="ps_x", tag="m")
                for k in range(4):
                    nc.tensor.matmul(ps_x[:], wo2_sb[:, k, :], relu[:, k, :],
                                     start=(k == 0), stop=(k == 3))
                # bf16 copy first: it feeds Wih0 on the critical path; the
                # f32 output copy and its DMA ride the scalar queue so the
                # sync queue stays clear for the latency-critical inb DMA
                nc.scalar.activation(ch.xT[:, :], ps_x[:], AF.Identity,
                                     bias=bo2_sb[:, 0:1])
                nc.scalar.activation(ch.xTf[:, :], ps_x[:], AF.Identity,
                                     bias=bo2_sb[:, 0:1])
                nc.scalar.dma_start(OUT.ap()[t][:, ch.b0:ch.b0 + NB],
                                    ch.xTf[:, :])
                if not last:
                    for g in (2, 0, 1, 3):
                        nc.tensor.matmul(g0n[g][:], wih0_sb[:, g, :],
                                         ch.xT[:, :], start=False, stop=True)
                    h1ch = lstm_nonlin(ch, g0n, ch.c1, bg0_sb, 0)
                    gather(ch, h1ch, ch.h1T, 0)

            # kick off step 0's layer-0 nonlinearity + h1 gathers
            for ch in chains:
                h1ch = lstm_nonlin(ch, ch.g0, ch.c1, bg0_sb, 0)
                gather(ch, h1ch, ch.h1T, 0)

            for t in range(seq):
                for ch in chains:
                    emit_front(ch, t)
                for ch in chains:
                    emit_back(ch, t)

    nc.compile()
    return nc


def _lhsT_tiles(W, rows, K):
    """W[rows] viewed as lhsT tiles: [128, MT, KTl, 128] with
    out[ki, mt, kt, mi] = W[rows[mt*128+mi], kt*128+ki]."""
    R = len(rows)
    MT = R // 128
    KTl = K // 128
    t = W[rows].reshape(MT, 128, KTl, 128)          # [mt, mi, kt, ki]
    return np.ascontiguousarray(t.transpose(3, 0, 2, 1)).astype(np.float32)


def _prep_inputs(inputs):
    import ml_dtypes
    np_dt = {"fp32": np.float32, "bf16": ml_dtypes.bfloat16,
             "fp32r": np.float32}[_mmdt()]
    f = lambda k: np.asarray(inputs[k], np.float32)
    W_proj, b_proj = f("W_proj"), f("b_proj")
    W_ih0, W_hh0 = f("W_ih0"), f("W_hh0")
    b_ih0, b_hh0 = f("b_ih0"), f("b_hh0")
    W_ih1, W_hh1 = f("W_ih1"), f("W_hh1")
    b_ih1, b_hh1 = f("b_ih1"), f("b_hh1")
    W_o1, b_o1 = f("W_o1"), f("b_o1")
    W_o2, b_o2 = f("W_o2"), f("b_o2")
    z = np.concatenate([f("z_primitive"), f("z_skill"), f("z_style")], axis=1)

    wo1 = _lhsT_tiles(W_o1, np.arange(512), H)
    wo2 = np.ascontiguousarray(
        W_o2.T.reshape(4, 128, 8).transpose(1, 0, 2)).astype(np.float32)
    bo1 = np.ascontiguousarray(b_o1.reshape(4, 128).T).astype(np.float32)
    bo2 = b_o2.reshape(8, 1).astype(np.float32)
    ztp = np.zeros((256, B), np.float32)
    ztp[:TOT] = z.T
    zt = np.ascontiguousarray(
        ztp.reshape(2, 128, B).transpose(1, 0, 2))
    Wp = np.zeros((2 * H, 256), np.float32)
    Wp[:, :TOT] = W_proj
    bias_g0 = b_ih0 + b_hh0
    bias_g1 = b_ih1 + b_hh1

    in_maps = []
    for c in range(N_CORES):
        rows_g = np.concatenate(
            [g * H + c * HC + np.arange(HC) for g in range(4)])
        rows_p = np.concatenate([np.arange(H), H + c * HC + np.arange(HC)])
        wih0 = np.ascontiguousarray(
            W_ih0[rows_g].reshape(4, 128, 8).transpose(2, 0, 1)).astype(
                np.float32)
        in_maps.append({
            "whh0": _lhsT_tiles(W_hh0, rows_g, H).astype(np_dt),
            "wih1": _lhsT_tiles(W_ih1, rows_g, H).astype(np_dt),
            "whh1": _lhsT_tiles(W_hh1, rows_g, H).astype(np_dt),
            "wo1": wo1.astype(np_dt),
            "wih0": wih0.astype(np_dt),
            "wo2": wo2.astype(np_dt),
            "wproj": _lhsT_tiles(Wp, rows_p, 256),
            "zt": zt,
            "bg0": np.ascontiguousarray(
                bias_g0[rows_g].reshape(4, 128).T).astype(np.float32),
            "bg1": np.ascontiguousarray(
                bias_g1[rows_g].reshape(4, 128).T).astype(np.float32),
            "bo1": bo1,
            "bo2": bo2,
            "bproj": np.ascontiguousarray(
                b_proj[rows_p].reshape(9, 128).T).astype(np.float32),
        })
    return in_maps


def kernel(**inputs):
    from concourse.bass_utils import run_bass_kernel_spmd

    seq = int(os.environ.get("BASS_KERNEL_SEQ", SEQ))
    key = (seq, _mmdt())
    if key not in _CACHE:
        _CACHE[key] = _build(seq, _mmdt())
    nc = _CACHE[key]
    in_maps = _prep_inputs(inputs)

    trace = os.environ.get("BASS_KERNEL_TRACE", "") == "1"
    kwargs = {}
    if trace:
        kwargs["trace"] = True
        kwargs["tmpdir"] = os.environ.get("BASS_KERNEL_TRACE_DIR") or None
    res = run_bass_kernel_spmd(nc, in_maps, core_ids=list(range(N_CORES)),
                               **kwargs)
    if trace:
        kernel.last_exec_time_ns = res.exec_time_ns
    out = res.results[0]["out"]          # [seq, 8, B]
    return np.ascontiguousarray(out.transpose(2, 0, 1)).astype(np.float32)


kernel.last_exec_time_ns = None

